# revision 13
# baseline (speedup 1.0000x reference)
import os
import sys
os.environ.setdefault("CONCOURSE_SCRUB_NEFF_DEBUG_INFO", "1")
if "/opt/trn_rl_repo" not in sys.path:
    sys.path.insert(0, "/opt/trn_rl_repo")
import hashlib
import numpy as np
import ml_dtypes
import jax
jax.config.update("jax_compilation_cache_dir", "/tmp/jax_kernel_cache")
jax.config.update("jax_persistent_cache_min_compile_time_secs", 0)
jax.config.update("jax_persistent_cache_min_entry_size_bytes", 0)
import concourse.bass as bass
from concourse import bacc
import concourse.tile as tile
from concourse import mybir
from concourse import masks as cmasks

F32 = mybir.dt.float32
F32R = mybir.dt.float32r
BF16 = mybir.dt.bfloat16
AF = mybir.ActivationFunctionType
ALU = mybir.AluOpType

D = 512
H = 8
HD = 64
L = 2
IN = 16
S = 1024
BL = 2          # batch elems per core
NCORES = 8
LN_EPS = 1e-5
DELTA_SCALE = 1.5
NEG = -1.0e30

# fwT slots: 0=x (query_w.T), 1=fused q0, 2=fused k0, 3=fused k1, 4=fused q1
FX, FQ0, FK0, FK1, FQ1 = range(5)
# smalls columns
XB, QB0F, KB0F, QB1, KB1F, OBP0, OBP1, LNG0, LNB0, LNG1, LNB1, OPW = \
    0, 4, 8, 12, 16, 20, 24, 28, 32, 36, 40, 44
OPB = 48
QB1B = 52
NSM = 56


def _build(consts, gates):
    nc = bacc.Bacc(None, target_bir_lowering=False, debug=False, num_devices=NCORES,
                   disable_frame_to_traceback=True)
    featT_e = nc.declare_dram_parameter("featT", [BL, IN, S], BF16, isOutput=False)
    out_e = nc.declare_dram_parameter("out", [BL, S], F32, isOutput=True)
    fwT_e = nc.inline_tensor(consts["fw"], name="fwT")
    w2n_e = nc.inline_tensor(consts["w2n"], name="w2n")
    w2qn_e = nc.inline_tensor(consts["w2qn"], name="w2qn")
    swn_e = nc.inline_tensor(consts["swn"], name="swn")
    sm_e = nc.inline_tensor(consts["smalls"], name="smalls")
    with tile.TileContext(nc) as tc:
        _emit(nc, tc, gates, dict(featT=featT_e, fwT=fwT_e, w2n=w2n_e,
                                  w2qn=w2qn_e, swn=swn_e, sm=sm_e, out=out_e))
    nc.compile()
    # scrub tracebacks and absolute source paths from debug info so the BIR
    # (and thus the HLO/NEFF cache keys) are independent of where this file
    # lives and who called it; also memoize the frozen serialization
    import orjson
    b = orjson.loads(nc.to_json_bytes())
    for e in b.get("debug_table", []):
        if isinstance(e, dict) and "ant_traceback" in e:
            e["ant_traceback"] = ""
    jb = orjson.dumps(b)
    for p in {__file__, os.path.abspath(__file__)}:
        jb = jb.replace(p.encode(), b"kernel.py")
    nc.to_json_bytes = (lambda _jb=jb: _jb)
    return nc


def _emit(nc, tc, gates, E):
    from contextlib import ExitStack
    ctx = ExitStack()
    with ctx:
        P = bass.MemorySpace.PSUM
        wp = ctx.enter_context(tc.tile_pool(name="wp", bufs=1))
        feat_p = ctx.enter_context(tc.tile_pool(name="feat", bufs=1))
        fa_p = ctx.enter_context(tc.tile_pool(name="fa", bufs=1))
        x_p = ctx.enter_context(tc.tile_pool(name="x", bufs=1))
        y_p = ctx.enter_context(tc.tile_pool(name="y", bufs=1))
        q_p = ctx.enter_context(tc.tile_pool(name="q", bufs=1))
        k_p = ctx.enter_context(tc.tile_pool(name="k", bufs=1))
        gt_p = ctx.enter_context(tc.tile_pool(name="gt", bufs=1))
        gs_p = ctx.enter_context(tc.tile_pool(name="gs", bufs=1))
        pr_p = ctx.enter_context(tc.tile_pool(name="pr", bufs=1))
        x2_p = ctx.enter_context(tc.tile_pool(name="x2", bufs=1))
        tmp_p = ctx.enter_context(tc.tile_pool(name="tmp", bufs=1))
        sinv_p = ctx.enter_context(tc.tile_pool(name="sinv", bufs=1))
        row_p = ctx.enter_context(tc.tile_pool(name="row", bufs=1))
        rs_p = ctx.enter_context(tc.tile_pool(name="rs", bufs=1))
        ms_p = ctx.enter_context(tc.tile_pool(name="ms", bufs=1))
        psA = ctx.enter_context(tc.tile_pool(name="psA", bufs=2, space=P))
        psS = ctx.enter_context(tc.tile_pool(name="psS", bufs=2, space=P))
        psV = ctx.enter_context(tc.tile_pool(name="psV", bufs=2, space=P))
        psB = ctx.enter_context(tc.tile_pool(name="psB", bufs=2, space=P))

        # ---- persistent weights/consts ----
        fwT = wp.tile([IN, 5, D], BF16)
        w2n = wp.tile([128, L, D], BF16)
        w2qn = wp.tile([128, D], BF16)
        swn = wp.tile([1, D], F32)
        sm = wp.tile([128, NSM], F32)
        ones64 = wp.tile([1, HD], F32)
        ones128c0 = wp.tile([128, 1], F32)
        ones128r0 = wp.tile([1, 128], F32)

        g = nc.gpsimd
        g.dma_start(fwT[:], E["fwT"][:])
        g.dma_start(w2n[:], E["w2n"][:])
        g.dma_start(w2qn[:], E["w2qn"][:])
        g.dma_start(swn[:], E["swn"][:])
        g.dma_start(sm[:], E["sm"][:])
        g.memset(ones64[:], 1.0)
        g.memset(ones128c0[:], 1.0)
        g.memset(ones128r0[:], 1.0)
        ones16r = ones64[0:1, 0:16].bitcast(F32R)
        ones128c = ones128c0[:].bitcast(F32R)
        ones128r = ones128r0[:].bitcast(F32R)

        maskA0 = wp.tile([128, 128], F32)
        g.memset(maskA0[:], 0.0)
        g.affine_select(out=maskA0[:], in_=maskA0[:],
                        compare_op=ALU.is_ge, fill=NEG, base=0,
                        pattern=[[1, 128]], channel_multiplier=-1)
        maskA = maskA0[:]
        identb = wp.tile([IN, IN], BF16)
        cmasks.make_identity(nc, identb[:])
        ident = identb[:]

        # faN cols 16:32 (zeros) and 32 (ones) are batch-invariant

        # ---- hoisted work tiles (reused across batches/layers) ----
        featT = feat_p.tile([IN, S], BF16)
        faN = fa_p.tile([128, 8, 33], BF16)
        g.memset(faN[:, :, IN:32], 0.0)
        g.memset(faN[:, :, 32], 1.0)
        xT = x_p.tile([128, 4, S], F32R)
        yT = y_p.tile([128, 4, S], F32R)
        qT = q_p.tile([128, 4, S], BF16)
        kT = k_p.tile([128, 4, S], BF16)
        gTA = gt_p.tile([128, S], BF16, name="gTA")
        gTB = gt_p.tile([128, S], BF16, name="gTB")
        rsave = rs_p.tile([128, 2, 512], F32)
        msave = ms_p.tile([1, 2, 512], F32R)
        gstg = [gs_p.tile([IN, 512], BF16, name=f"gstg{i}") for i in range(2)]
        probt = [pr_p.tile([128, 8, 512], BF16, name=f"probt{i}") for i in range(2)]
        x2t = [x2_p.tile([128, 512], F32R, name=f"x2t{i}") for i in range(2)]
        tmpt = [tmp_p.tile([128, 512], F32, name=f"tmpt{i}") for i in range(2)]
        sinvt = [sinv_p.tile([IN, 512], F32, name=f"sinvt{i}") for i in range(2)]
        srowt = [row_p.tile([1, 512], F32R, name=f"srowt{i}") for i in range(2)]
        rowt = [row_p.tile([1, 512], F32, name=f"rowt{i}") for i in range(2)]
        rowr = row_p.tile([1, 512], F32R, name="rowr")
        psa = [psA.tile([128, 512], F32, tag="a", name=f"psa{i}") for i in range(2)]
        pss = [psS.tile([128, 512], F32, tag="s", name=f"pss{i}") for i in range(2)]
        psv = [psV.tile([33, 512], F32, tag="v", name=f"psv{i}") for i in range(2)]
        psb = [psB.tile([128, 512], F32, tag="b", name=f"psb{i}") for i in range(2)]

        for b in range(BL):
            g.dma_start(featT[:], E["featT"][b])

            # features in natural layout [t, c] + ones col at 32 (PSUM
            # partition-32 alignment for the softmax-sum row), zeros 16:32
            for tt in range(8):
                ps = psa[tt % 2]
                nc.tensor.matmul(ps[:, 0:IN], featT[:, tt * 128:(tt + 1) * 128],
                                 ident, start=True, stop=True)
                nc.scalar.copy(faN[:, tt, 0:IN], ps[:, 0:IN])

            # residual stream x = features @ query_w.T + query_b
            for dt in range(4):
                for qs in range(2):
                    cols = bass.ts(qs, 512)
                    ps = psa[qs]
                    nc.tensor.matmul(ps[:], fwT[:, FX, dt * 128:(dt + 1) * 128],
                                     featT[:, cols], start=True, stop=True)
                    nc.scalar.activation(xT[:, dt, cols], ps[:], AF.Identity,
                                         bias=sm[:, XB + dt:XB + dt + 1])

            for l in range(L):
                gT = gTA if l == 0 else gTB
                # ---- q projection (transposed layout, bf16 out) ----
                if l == 0:
                    for dt in range(4):
                        for qs in range(2):
                            cols = bass.ts(qs, 512)
                            ps = psa[qs]
                            nc.tensor.matmul(
                                ps[:], fwT[:, FQ0, dt * 128:(dt + 1) * 128],
                                featT[:, cols], start=True, stop=True)
                            nc.scalar.activation(
                                qT[:, dt, cols], ps[:], AF.Identity,
                                bias=sm[:, QB0F + dt:QB0F + dt + 1])
                else:
                    # q1 = rstd * (F@fq1z + G0@w2qn - mu (x) sw + cY) + bias
                    for dt in range(4):
                        for qs in range(2):
                            cols = bass.ts(qs, 512)
                            ps = psa[qs]
                            nc.tensor.matmul(
                                ps[:], fwT[:, FQ1, dt * 128:(dt + 1) * 128],
                                featT[:, cols], start=True, stop=False)
                            nc.tensor.matmul(
                                ps[:], w2qn[:, dt * 128:(dt + 1) * 128],
                                gTA[:, cols], start=False, stop=False)
                            nc.tensor.matmul(
                                ps[:], swn[0:1, dt * 128:(dt + 1) * 128].bitcast(F32R),
                                msave[0:1, qs, :], start=False, stop=True)
                            nc.vector.scalar_tensor_tensor(
                                qT[:, dt, cols], ps[:],
                                sm[:, QB1 + dt:QB1 + dt + 1],
                                rsave[:, qs, :], op0=ALU.add, op1=ALU.mult)
                            if gates["qb1b"]:
                                nc.vector.tensor_scalar_add(
                                    qT[:, dt, cols], qT[:, dt, cols],
                                    sm[:, QB1B + dt:QB1B + dt + 1])

                # ---- k projection (fused rank-16, both layers) ----
                fk = FK0 if l == 0 else FK1
                kbc = KB0F if l == 0 else KB1F
                for dt in range(4):
                    for qs in range(2):
                        cols = bass.ts(qs, 512)
                        ps = psa[qs]
                        nc.tensor.matmul(ps[:], fwT[:, fk, dt * 128:(dt + 1) * 128],
                                         featT[:, cols], start=True, stop=True)
                        nc.scalar.activation(kT[:, dt, cols], ps[:], AF.Identity,
                                             bias=sm[:, kbc + dt:kbc + dt + 1])

                # ---- attention: G_h = softmax(qk) @ [F|1] per head ----
                for h in range(H):
                    hp = (h % 2) * 64
                    dht = h // 2
                    for qblk in range(2):
                        i2 = (h * 2 + qblk) % 2
                        probsT = probt[i2]
                        pv = psv[i2]
                        nkj = 4 * (qblk + 1)
                        for kj in range(nkj):
                            off = max(0, (kj - 4 * qblk) * 128)
                            sc = pss[kj % 2]
                            nc.tensor.matmul(
                                sc[:, off:],
                                kT[hp:hp + 64, dht, kj * 128:(kj + 1) * 128],
                                qT[hp:hp + 64, dht, qblk * 512 + off:(qblk + 1) * 512],
                                start=True, stop=True)
                            if kj >= 4 * qblk:
                                nc.vector.tensor_add(sc[:, off:off + 128],
                                                     sc[:, off:off + 128], maskA)
                            nc.scalar.activation(probsT[:, kj, off:], sc[:, off:],
                                                 AF.Exp, scale=0.125)
                            nc.tensor.matmul(pv[:, off:], faN[:, kj, :],
                                             probsT[:, kj, off:],
                                             start=(kj == 0), stop=(kj == nkj - 1))
                        # normalize by softmax sums (row 32 of pv), stage in a
                        # [16,512] tile, then DMA into gT's h*16 partition slot
                        # (DMA has no 32-partition alignment restriction)
                        srow = srowt[i2]
                        nc.scalar.copy(srow[:], pv[32:33, :])
                        sb = psb[i2][0:IN, :]
                        nc.tensor.matmul(sb, ones16r, srow[:],
                                         start=True, stop=True)
                        sinv = sinvt[i2]
                        nc.vector.reciprocal(sinv[:], sb)
                        cols = bass.ts(qblk, 512)
                        stg = gstg[i2]
                        nc.vector.tensor_mul(stg[:], pv[0:IN, :], sinv[:])
                        nc.sync.dma_start(gT[h * IN:(h + 1) * IN, cols], stg[:])

                # ---- attn out = W2.T @ G (+obp) + residual add ----
                obc = OBP0 if l == 0 else OBP1
                for dt in range(4):
                    for qs in range(2):
                        cols = bass.ts(qs, 512)
                        ps = psa[qs]
                        nc.tensor.matmul(
                            ps[:], w2n[:, l, dt * 128:(dt + 1) * 128],
                            gT[:, cols], start=True, stop=True)
                        nc.vector.scalar_tensor_tensor(
                            yT[:, dt, cols], ps[:], sm[:, obc + dt:obc + dt + 1],
                            xT[:, dt, cols].bitcast(F32),
                            op0=ALU.add, op1=ALU.add)

                # ---- layernorm ----
                lngc = LNG0 if l == 0 else LNG1
                lnbc = LNB0 if l == 0 else LNB1
                for qs in range(2):
                    cols = bass.ts(qs, 512)
                    mps = pss[0][0:1, :]
                    for dt in range(4):
                        nc.tensor.matmul(mps, ones128c, yT[:, dt, cols],
                                         start=(dt == 0), stop=(dt == 3))
                    vps = pss[1][0:1, :]
                    for dt in range(4):
                        x2 = x2t[dt % 2]
                        nc.scalar.activation(x2[:], yT[:, dt, cols].bitcast(F32),
                                             AF.Square)
                        nc.tensor.matmul(vps, ones128c, x2[:],
                                         start=(dt == 0), stop=(dt == 3))
                    mrow = msave[0:1, qs, :]
                    nc.scalar.mul(mrow, mps, 1.0 / D)
                    s1 = rowt[0]
                    nc.vector.tensor_mul(s1[:], mrow.bitcast(F32),
                                         mrow.bitcast(F32))
                    s2 = rowt[1]
                    nc.vector.scalar_tensor_tensor(
                        s2[:], vps, 1.0 / D, s1[:],
                        op0=ALU.mult, op1=ALU.subtract)
                    s4 = rowt[0]
                    nc.vector.tensor_scalar_add(s4[:], s2[:], LN_EPS)
                    s3 = rowt[1]
                    nc.scalar.sqrt(s3[:], s4[:])
                    with nc.allow_low_precision(reason="f32r rstd"):
                        nc.vector.reciprocal(rowr[:], s3[:])
                    # broadcast mean and rstd to 128 partitions
                    mbps = psb[0]
                    nc.tensor.matmul(mbps[:], ones128r, mrow,
                                     start=True, stop=True)
                    mbc = mbps[:]
                    rbps = psb[1]
                    nc.tensor.matmul(rbps[:], ones128r, rowr[:],
                                     start=True, stop=True)
                    if l == 0:
                        rbc = rsave[:, qs, :]
                        nc.scalar.copy(rbc, rbps[:])
                    else:
                        rbc = rbps[:]
                    for dt in range(4):
                        tmp = tmpt[dt % 2]
                        nc.vector.tensor_sub(tmp[:], yT[:, dt, cols].bitcast(F32),
                                             mbc)
                        nc.vector.scalar_tensor_tensor(
                            xT[:, dt, cols], tmp[:], sm[:, lngc + dt:lngc + dt + 1],
                            rbc, op0=ALU.mult, op1=ALU.mult)
                        if gates["lnb"]:
                            nc.vector.tensor_scalar_add(
                                xT[:, dt, cols], xT[:, dt, cols],
                                sm[:, lnbc + dt:lnbc + dt + 1])

            # ---- final projection + tanh ----
            for qs in range(2):
                cols = bass.ts(qs, 512)
                fps = pss[0][0:1, :]
                for dt in range(4):
                    nc.tensor.matmul(fps,
                                     sm[:, OPW + dt:OPW + dt + 1].bitcast(F32R),
                                     xT[:, dt, cols], start=(dt == 0),
                                     stop=(dt == 3))
                th = rowt[0]
                nc.scalar.activation(th[:], fps, AF.Tanh,
                                     bias=sm[0:1, OPB:OPB + 1])
                orow = rowt[1]
                nc.scalar.mul(orow[:], th[:], DELTA_SCALE)
                nc.sync.dma_start(E["out"][b:b + 1, cols], orow[:])


def _host_pack(inputs):
    f32 = np.float32
    ip = {k: np.asarray(v, f32) for k, v in inputs.items()}
    featT = np.ascontiguousarray(
        ip["features"].transpose(0, 2, 1).astype(ml_dtypes.bfloat16))  # [B, IN, S]

    hw, hb = ip["hist_w"], ip["hist_b"]
    qw, qb = ip["query_w"], ip["query_b"]
    ipw, ipb = ip["in_proj_w"], ip["in_proj_b"]
    wo, ob = ip["attn_out_w"], ip["attn_out_b"]
    wq, wk, wv = ipw[:, :D], ipw[:, D:2 * D], ipw[:, 2 * D:]
    bq, bk, bv = ipb[:, :D], ipb[:, D:2 * D], ipb[:, 2 * D:]

    g1 = ip["ln_g"][0]
    wq1g = wq[1] * g1[None, :]      # fold LN0 gain into wq1 input dims
    fw = np.stack([
        qw.T,               # x residual stream
        (wq[0] @ qw).T,     # fused q layer 0
        (wk[0] @ hw).T,     # fused k layer 0
        (wk[1] @ hw).T,     # fused k layer 1
        (wq1g @ qw).T,      # fused q layer 1 (F term)
    ], axis=1).astype(ml_dtypes.bfloat16)  # [16, 5, 512]

    # W2_l[h*16+c, d] = (wo_l[:, h-dims] @ (wv_l @ hw)[h-dims, :])[d, c]
    def w2(l):
        fvw = wv[l] @ hw  # [512, 16]
        out = np.zeros((128, D), f32)
        for h in range(H):
            out[h * IN:(h + 1) * IN] = (wo[l][:, h * HD:(h + 1) * HD]
                                        @ fvw[h * HD:(h + 1) * HD, :]).T
        return out

    w20, w21 = w2(0), w2(1)
    w2n = np.stack([w20, w21], axis=1).astype(ml_dtypes.bfloat16)  # [128, L, D]
    w2qn = (w20 @ wq1g.T).astype(ml_dtypes.bfloat16)  # [128, D]
    swn = -wq1g.sum(axis=1).astype(f32).reshape(1, D)  # -sw for the mu term

    def colpack(v):  # [512] -> [128, 4]
        return np.asarray(v, f32).reshape(4, 128).T

    vb0f = hb @ wv[0].T + bv[0]
    vb1f = hb @ wv[1].T + bv[1]
    sm = np.zeros((128, NSM), f32)
    sm[:, XB:XB + 4] = colpack(qb)
    sm[:, QB0F:QB0F + 4] = colpack(qb @ wq[0].T + bq[0])
    sm[:, KB0F:KB0F + 4] = colpack(hb @ wk[0].T + bk[0])
    obp0_full = vb0f @ wo[0].T + ob[0]
    sm[:, QB1:QB1 + 4] = colpack((qb + obp0_full) @ wq1g.T)
    qb1b = ip["ln_b"][0] @ wq[1].T + bq[1]
    sm[:, QB1B:QB1B + 4] = colpack(qb1b)
    sm[:, KB1F:KB1F + 4] = colpack(hb @ wk[1].T + bk[1])
    sm[:, OBP0:OBP0 + 4] = colpack(obp0_full)
    sm[:, OBP1:OBP1 + 4] = colpack(vb1f @ wo[1].T + ob[1])
    sm[:, LNG0:LNG0 + 4] = colpack(ip["ln_g"][0])
    sm[:, LNB0:LNB0 + 4] = colpack(ip["ln_b"][0])
    sm[:, LNG1:LNG1 + 4] = colpack(ip["ln_g"][1])
    sm[:, LNB1:LNB1 + 4] = colpack(ip["ln_b"][1])
    sm[:, OPW:OPW + 4] = colpack(ip["out_proj_w"][0])
    sm[0, OPB] = ip["out_proj_b"][0]
    sm = sm.astype(f32)

    consts = dict(fw=fw, w2n=np.ascontiguousarray(w2n),
                  w2qn=np.ascontiguousarray(w2qn), swn=swn,
                  smalls=np.ascontiguousarray(sm))
    gates = dict(lnb=bool(np.any(ip["ln_b"] != 0.0)),
                 qb1b=bool(np.any(qb1b != 0.0)))
    return featT, consts, gates


_CACHE = {}


def build_and_inmaps(inputs):
    featT, consts, gates = _host_pack(inputs)
    hsh = hashlib.sha1()
    for k in sorted(consts):
        hsh.update(consts[k].tobytes())
    key = (hsh.hexdigest(), gates["lnb"], gates["qb1b"])
    if key not in _CACHE:
        _CACHE[key] = _build(consts, gates)
    nc = _CACHE[key]
    in_maps = [
        {"featT": np.ascontiguousarray(featT[c * BL:(c + 1) * BL])}
        for c in range(NCORES)
    ]
    return nc, in_maps


def kernel(**inputs):
    from concourse.bass_utils import run_bass_kernel_spmd
    nc, in_maps = build_and_inmaps(inputs)
    res = run_bass_kernel_spmd(nc, in_maps, list(range(NCORES)))
    outs = [res.results[c]["out"] for c in range(NCORES)]
    return np.concatenate(outs, axis=0).astype(np.float32)
